# revision 4
# baseline (speedup 1.0000x reference)
"""ContentPhasorStream kernel for 8 Trainium2 NeuronCores — single launch.

Math: the reference is causal linear attention with feature map
[cos(phase), sin(phase)] (2K = 64 features):

  retrieved[b,l,d] = sum_{t<=l} v[b,t,d] * sum_k cos(qp[b,l,k] - kp[b,t,k])
                   = Qf[l] @ cumsum_t(Kf[t]^T v[t])      (Qf/Kf = [cos, sin] feats)

so the [B,L,K,D] cumsum never needs to be materialized.  Chunked scan
(128-position chunks): per chunk O = mask(Qf Kf^T) V + Qf S with S += Kf^T V.

Sharding: 8 cores = 2 batches x 4 L-segments of 512.  Each core computes the
MLPs + intra-segment attention + its segment summary state S_seg [64,256];
the cross-segment prefix is resolved IN-KERNEL with an AllGather of S_seg
over each batch's 4-core group (tiny, ~64KB, runs on TOPSP/SDMA and overlaps
with the intra-segment attention), followed by a per-core 0/1-weighted sum
and a correction matmul O += Qf @ S_prefix.  One kernel launch total.

Perf structure:
 - all matmul operands in bf16 (fp32 PSUM accumulation): halves input DMA,
   and small-moving-dim attention matmuls run at 1 cyc/row at any clock
 - k|q phases packed in one [128,512] tile: one tanh + one sin instruction
   (2 ACT table loads total, no thrash); cos rows take |t| via DVE sign mask
 - output scaling on DVE (ACT stays on the tanh/sin chain)
 - scratch warmup matmuls ramp the PE clock (HAM) during the DMA wait
"""

import math
import sys

import numpy as np

for _p in ("/opt/trn_rl_repo", "/root/.axon_site/_ro/trn_rl_repo"):
    if _p not in sys.path:
        sys.path.insert(0, _p)

# bass_utils imports antenv.axon_hooks when BASS_TRACE is set; provide a
# no-op registry if the image's antenv lacks that module so tracing degrades
# gracefully instead of crashing.
try:
    import antenv.axon_hooks  # noqa: F401
except Exception:
    import types as _types

    _ah = _types.ModuleType("antenv.axon_hooks")
    _ah._HOOK = None

    def _set_hook(h, _m=_ah):
        _m._HOOK = h

    _ah.set_axon_ntff_profile_hook = _set_hook
    _ah.get_axon_ntff_profile_hook = lambda _m=_ah: _m._HOOK
    sys.modules["antenv.axon_hooks"] = _ah

import concourse.bass as bass
import concourse.mybir as mybir
import concourse.tile as tile
from concourse import bacc
from concourse.bass_utils import run_bass_kernel_spmd
from concourse.masks import make_upper_triangular

B, L, D, NK = 2, 2048, 256, 32
NCORES = 8
NSEG = NCORES // B          # 4 segments per batch
SEG = L // NSEG             # 512 positions per core
CH = 128                    # attention chunk
NCH = SEG // CH             # 4 chunks per segment
F = 2 * NK                  # 64 = [cos, sin] feature dim
PI = math.pi
NWARM = 7                   # PE clock-ramp matmuls
NFILL = 6                   # PE keep-warm fillers over the feature-wait gap

F32 = mybir.dt.float32
BF16 = mybir.dt.bfloat16
U32 = mybir.dt.uint32
NPBF16 = mybir.dt.np(BF16)
TANH = mybir.ActivationFunctionType.Tanh
SIN = mybir.ActivationFunctionType.Sin
PSUM = bass.MemorySpace.PSUM
AND = mybir.AluOpType.bitwise_and
MULT = mybir.AluOpType.mult
ADD = mybir.AluOpType.add

# cpf (fp32 constant pack) column layout
CPF_B1K = 0           # [128, 2]
CPF_B1Q = 2           # [128, 2]
CPF_B2 = 4            # [128, 1] packed doubled phase bias  [kb2;kb2;qb2;qb2]
CPF_PSC = 5           # [128, 1] sin scale  [-pi, +pi, -pi, +pi] x32
CPF_PBI = 6           # [128, 1] sin bias   [pi/2, 0, pi/2, 0] x32
CPF_NRM = 7           # [128, 4] per-chunk 1/sqrt((pos+1)K)
CPF_W = 11            # [64, 4]  per-core prefix 0/1 weights
CPF_N = 15

RGROUPS = [[0, 1, 2, 3], [4, 5, 6, 7]]


def _build_kernel():
    nc = bacc.Bacc("TRN2", target_bir_lowering=False, debug=False)

    w1k_d = nc.dram_tensor("w1k", [128, 2, D], BF16, kind="ExternalInput").ap()
    xt_d = nc.dram_tensor("xt", [128, 2, SEG], BF16, kind="ExternalInput").ap()
    wqv_d = nc.dram_tensor("wqv", [128, 4, D], BF16, kind="ExternalInput").ap()
    cpr_d = nc.dram_tensor("cpr", [128, 5, F], BF16, kind="ExternalInput").ap()
    vbb_d = nc.dram_tensor("vbb", [128, D], BF16, kind="ExternalInput").ap()
    cpf_d = nc.dram_tensor("cpf", [128, CPF_N], F32, kind="ExternalInput").ap()

    o_d = nc.dram_tensor("o", [SEG, D], F32, kind="ExternalOutput").ap()

    with tile.TileContext(nc) as tc:
        with (
            tc.tile_pool(name="const", bufs=1) as constp,
            tc.tile_pool(name="hsb", bufs=4) as hsbp,
            tc.tile_pool(name="work", bufs=2) as workp,
            tc.tile_pool(name="dram", bufs=1, space="DRAM") as dramp,
        ):
            # ---- PE warmup on scratch data (ramp HAM during DMA wait) ----
            warm_sb = constp.tile([128, SEG], BF16)
            nc.vector.memset(warm_sb[:].bitcast(mybir.dt.uint16), 0)
            # preload the tanh LUT off the critical chain while DMAs are in
            # flight (each tanh<->sin switch reloads the single table slot,
            # so only the first function is worth preloading)
            lutw = constp.tile([1, 1], F32)
            nc.scalar.activation(lutw[:], warm_sb[0:1, 0:1], TANH)

            # ---- packed input DMAs, two HWDGE rings, first-needed first ----
            cpf_sb = constp.tile([128, CPF_N], F32)
            nc.sync.dma_start(cpf_sb[:], cpf_d[:, :])
            cpr_sb = constp.tile([128, 5, F], BF16)
            nc.scalar.dma_start(cpr_sb[:], cpr_d[:, :, :])
            w1k_sb = constp.tile([128, 2, D], BF16)
            nc.sync.dma_start(w1k_sb[:], w1k_d[:, :, :])
            xt_sb = constp.tile([128, 2, SEG], BF16)
            nc.scalar.dma_start(xt_sb[:, 1, :], xt_d[:, 1, :])
            nc.sync.dma_start(xt_sb[:, 0, :], xt_d[:, 0, :])
            wqv_sb = constp.tile([128, 4, D], BF16)
            nc.scalar.dma_start(wqv_sb[:], wqv_d[:, :, :])
            vbb_sb = constp.tile([128, D], BF16)
            nc.sync.dma_start(vbb_sb[:], vbb_d[:, :])

            b1_ap = {"k": cpf_sb[:, CPF_B1K:CPF_B1K + 2],
                     "q": cpf_sb[:, CPF_B1Q:CPF_B1Q + 2]}
            b2_ap = cpf_sb[:, CPF_B2:CPF_B2 + 1]
            psc_ap = cpf_sb[:, CPF_PSC:CPF_PSC + 1]
            pbi_ap = cpf_sb[:, CPF_PBI:CPF_PBI + 1]
            nrm_ap = cpf_sb[:, CPF_NRM:CPF_NRM + NCH]
            w_ap = cpf_sb[0:F, CPF_W:CPF_W + NSEG]
            w2_idx = {"k": (0, 1), "q": (2, 3)}
            ident_ap = cpr_sb[0:F, 4, :]

            mask_sb = constp.tile([CH, CH], F32)
            make_upper_triangular(nc, mask_sb[:], val=1.0, diag=True)

            # collective bounce buffers (internal DRAM)
            sbounce = dramp.tile([F, D], F32)
            gbounce = dramp.tile([NSEG, F, D], F32)

            # ---- MLPs ----
            feat_sb = constp.tile([128, SEG], BF16)   # rows 0:64 Kf, 64:128 Qf
            with (
                tc.tile_pool(name="ph", bufs=4, space=PSUM) as php,
                tc.tile_pool(name="pp", bufs=1, space=PSUM) as ppp,
                tc.tile_pool(name="pv", bufs=2, space=PSUM) as pvp,
            ):
                for _ in range(NWARM):
                    pwarm = php.tile([128, SEG], F32, tag="ph")
                    nc.tensor.matmul(
                        pwarm[:], warm_sb[:, 0:128], warm_sb[:],
                        start=True, stop=True,
                    )

                # h = tanh(x W1 + b1), transposed layout [e, l]
                h_halves = {"k": [], "q": []}
                for name in ("k", "q"):
                    w1sb = w1k_sb if name == "k" else wqv_sb
                    for eh in range(2):
                        ph = php.tile([128, SEG], F32, tag="ph")
                        for dc in range(2):
                            nc.tensor.matmul(
                                ph[:],
                                w1sb[:, dc, eh * 128:(eh + 1) * 128],
                                xt_sb[:, dc, :],
                                start=(dc == 0),
                                stop=(dc == 1),
                            )
                        h_sb = hsbp.tile([128, SEG], BF16, tag="h")
                        nc.scalar.activation(
                            h_sb[:], ph[:], TANH, bias=b1_ap[name][:, eh:eh + 1]
                        )
                        h_halves[name].append(h_sb)

                # V chunks early: PE gap-filler while ACT runs tanh
                v_sbs = []
                for c in range(NCH):
                    cs = slice(c * CH, (c + 1) * CH)
                    pv = pvp.tile([CH, D], F32, tag="pv")
                    for dc in range(2):
                        nc.tensor.matmul(
                            pv[:],
                            xt_sb[:, dc, cs],
                            wqv_sb[:, 2 + dc, :],
                            start=(dc == 0),
                            stop=(dc == 1),
                        )
                    v_sb = workp.tile([CH, D], BF16, tag=f"v{c}")
                    nc.vector.tensor_add(v_sb[:], pv[:], vbb_sb[:])
                    v_sbs.append(v_sb)

                # packed phase t = tanh(h W2d + b2d): rows 0:64 k, 64:128 q
                pp = ppp.tile([128, SEG], F32, tag="pp")
                for name in ("k", "q"):
                    r0 = 0 if name == "k" else F
                    for eh in range(2):
                        nc.tensor.matmul(
                            pp[r0:r0 + F, :],
                            cpr_sb[:, w2_idx[name][eh], :],
                            h_halves[name][eh][:],
                            start=(eh == 0),
                            stop=(eh == 1),
                            skip_group_check=True,
                        )

                # keep-warm fillers: PE work covering the tanh/sin chain so
                # HAM stays at 2.4 GHz into the attention phase
                for _ in range(NFILL):
                    pwarm = php.tile([128, SEG], F32, tag="ph")
                    nc.tensor.matmul(
                        pwarm[:], warm_sb[:, 0:128], warm_sb[:],
                        start=True, stop=True,
                    )

                tk = workp.tile([128, SEG], F32, tag="tk")
                nc.scalar.activation(tk[:], pp[:], TANH, bias=b2_ap)

                # cos rows (0:32 k, 64:96 q) need |t|: clear sign bit on DVE
                for r0 in (0, F):
                    tku = tk[r0:r0 + NK, :].bitcast(U32)
                    nc.vector.tensor_scalar(
                        tku, tku, 0x7FFFFFFF, None, op0=AND
                    )
                # rows 0:32: sin(pi/2 - pi|t|) = cos; rows 32:64: sin(pi t);
                # same packing for q in rows 64:128 — one SIN instruction
                nc.scalar.activation(
                    feat_sb[:], tk[:], SIN, bias=pbi_ap, scale=psc_ap
                )
            # matmul needs Kf/Qf on the SAME partitions to contract over f;
            # shift Qf rows 64:128 down to a base-0 tile (SBUF-SBUF DMA is
            # the only engine that can move data across partitions cheaply)
            fq_sb = constp.tile([F, SEG], BF16)
            nc.sync.dma_start(fq_sb[:], feat_sb[F:128, :])
            fk = feat_sb[0:F, :]
            fq = fq_sb[:, :]

            # ---- chunked causal linear attention ----
            with (
                tc.tile_pool(name="po", bufs=2, space=PSUM) as pop,
                tc.tile_pool(name="pat", bufs=2, space=PSUM) as patp,
                tc.tile_pool(name="ptr", bufs=2, space=PSUM) as ptrp,
                tc.tile_pool(name="ps", bufs=1, space=PSUM) as psp,
            ):
                ps_tile = psp.tile([F, D], F32)
                state = {"s": None}

                # software pipeline, 2 chunks deep: AT/transpose of chunk c+1
                # issue before AV/state of chunk c, so the DVE mask/copy
                # latency hides under PE work and HAM stays warm
                def emit_front(c):
                    cs = slice(c * CH, (c + 1) * CH)
                    # A^T[t,l] = sum_f Kf[t,f] Qf[l,f]
                    pat = patp.tile([CH, CH], F32, tag="pat")
                    nc.tensor.matmul(
                        pat[:], fk[:, cs], fq[:, cs], start=True, stop=True,
                    )
                    ptr = ptrp.tile([CH, F], BF16, tag="ptr")
                    nc.tensor.transpose(ptr[:], fk[:, cs], ident_ap)
                    atm = workp.tile([CH, CH], BF16, tag="atm")
                    nc.vector.tensor_mul(atm[:], pat[:], mask_sb[:])
                    kf_sb = workp.tile([CH, F], BF16, tag="kf")
                    nc.vector.tensor_copy(kf_sb[:], ptr[:])
                    return atm, kf_sb

                def emit_back(c, atm, kf_sb):
                    cs = slice(c * CH, (c + 1) * CH)
                    # O_intra = (A V) [+ Qf S_prev]
                    po = pop.tile([CH, D], F32, tag="po")
                    nc.tensor.matmul(
                        po[:], atm[:], v_sbs[c][:], start=True, stop=(c == 0)
                    )
                    if c > 0:
                        nc.tensor.matmul(
                            po[:], fq[:, cs], state["s"][:],
                            start=False, stop=True,
                        )
                    # state S += Kf^T V
                    nc.tensor.matmul(
                        ps_tile[:], kf_sb[:], v_sbs[c][:],
                        start=(c == 0), stop=True, skip_group_check=True,
                    )
                    # scale O_intra by 1/sqrt((pos+1)K) on DVE, keep in SBUF
                    o_sb = workp.tile([CH, D], F32, tag=f"o{c}")
                    nc.vector.tensor_scalar(
                        o_sb[:], po[:], nrm_ap[:, c:c + 1], None, op0=MULT
                    )
                    if c < NCH - 1:
                        s_new = workp.tile([F, D], BF16, tag="s")
                        nc.vector.tensor_copy(s_new[:], ps_tile[:])
                        state["s"] = s_new
                    else:
                        s_f32 = workp.tile([F, D], F32, tag="sf")
                        nc.vector.tensor_copy(s_f32[:], ps_tile[:])
                        state["sf"] = s_f32
                    return o_sb

                o_sbs = []
                frontc = emit_front(0)
                for c in range(1, NCH):
                    nxt = emit_front(c)
                    o_sbs.append(emit_back(c - 1, *frontc))
                    frontc = nxt
                o_sbs.append(emit_back(NCH - 1, *frontc))

                # ---- cross-segment state exchange (overlaps with nothing
                # left on PE; runs on TOPSP/SDMA) ----
                nc.gpsimd.dma_start(sbounce[:], state["sf"][:])
                nc.gpsimd.collective_compute(
                    "AllGather",
                    mybir.AluOpType.bypass,
                    replica_groups=RGROUPS,
                    ins=[sbounce.opt()],
                    outs=[gbounce.opt()],
                )
                g_sb = constp.tile([F, NSEG, D], F32)
                for j in range(NSEG):
                    nc.sync.dma_start(g_sb[:, j, :], gbounce[j])

                # S_prefix = sum_j w_j * S_j  (w baked per core, 0/1)
                spre_f = workp.tile([F, D], F32, tag="spre_f")
                nc.vector.tensor_scalar(
                    spre_f[:], g_sb[:, 0, :], w_ap[:, 0:1], None, op0=MULT
                )
                for j in range(1, NSEG - 1):
                    nc.vector.scalar_tensor_tensor(
                        spre_f[:], g_sb[:, j, :], w_ap[:, j:j + 1], spre_f[:],
                        op0=MULT, op1=ADD,
                    )
                spre = workp.tile([F, D], BF16, tag="spre")
                nc.vector.scalar_tensor_tensor(
                    spre[:], g_sb[:, NSEG - 1, :], w_ap[:, NSEG - 1:NSEG],
                    spre_f[:], op0=MULT, op1=ADD,
                )

                # correction O += Qf @ S_prefix, then write out
                for c in range(NCH):
                    cs = slice(c * CH, (c + 1) * CH)
                    p2 = pop.tile([CH, D], F32, tag="po")
                    nc.tensor.matmul(
                        p2[:], fq[:, cs], spre[:], start=True, stop=True,
                    )
                    out_sb = workp.tile([CH, D], F32, tag=f"r{c}")
                    nc.vector.scalar_tensor_tensor(
                        out_sb[:], p2[:], nrm_ap[:, c:c + 1], o_sbs[c][:],
                        op0=MULT, op1=ADD,
                    )
                    nc.scalar.dma_start(o_d[cs, :], out_sb[:])

    nc.compile()
    return nc


_NC = None


def _get_nc():
    global _NC
    if _NC is None:
        _NC = _build_kernel()
    return _NC


def _split_heads(wt, dtype):
    # [256, N] row-chunked to [128, 2, N]
    n = wt.shape[1]
    return np.ascontiguousarray(
        wt.reshape(2, 128, n).transpose(1, 0, 2)
    ).astype(dtype)


def _in_maps(x, kw1, kb1, kw2, kb2, qw1, qb1, qw2, qb2, vw, vb):
    f32 = np.float32
    w1k = _split_heads(kw1.T, NPBF16)                  # [128, 2, 256]
    wqv = np.concatenate(
        [_split_heads(qw1.T, NPBF16), _split_heads(vw.T, NPBF16)], axis=1
    )                                                  # [128, 4, 256]
    kw2dT = np.vstack([kw2, kw2]).T                    # [256, 64]
    qw2dT = np.vstack([qw2, qw2]).T
    idn = np.zeros((128, F), dtype=f32)
    idn[:F] = np.eye(F, dtype=f32)
    cpr = np.concatenate(
        [_split_heads(kw2dT, NPBF16), _split_heads(qw2dT, NPBF16),
         idn[:, None, :].astype(NPBF16)], axis=1
    )                                                  # [128, 5, 64]
    vbb = np.broadcast_to(vb[None, :], (128, D)).astype(NPBF16)

    cpf = np.zeros((128, CPF_N), dtype=f32)
    cpf[:, CPF_B1K:CPF_B1K + 2] = kb1.reshape(2, 128).T
    cpf[:, CPF_B1Q:CPF_B1Q + 2] = qb1.reshape(2, 128).T
    cpf[:, CPF_B2] = np.concatenate([kb2, kb2, qb2, qb2])
    cpf[:, CPF_PSC] = np.concatenate(
        [np.full(NK, -PI), np.full(NK, PI)] * 2)
    cpf[:, CPF_PBI] = np.concatenate(
        [np.full(NK, PI / 2), np.zeros(NK)] * 2)

    in_maps = []
    for core in range(NCORES):
        b, s = divmod(core, NSEG)
        seg0 = s * SEG
        pos = seg0 + np.arange(SEG, dtype=np.float64) + 1.0
        nrm = (1.0 / np.sqrt(pos * NK)).astype(f32).reshape(NCH, CH).T
        cpf_c = cpf.copy()
        cpf_c[:, CPF_NRM:CPF_NRM + NCH] = nrm
        for j in range(NSEG):
            cpf_c[0:F, CPF_W + j] = 1.0 if j < s else 0.0
        in_maps.append({
            "w1k": w1k,
            "xt": _split_heads(
                np.ascontiguousarray(x[b, seg0:seg0 + SEG, :].T), NPBF16
            ),
            "wqv": wqv,
            "cpr": cpr,
            "vbb": vbb,
            "cpf": cpf_c,
        })
    return in_maps


LAST_RESULTS = []  # [BassKernelResults] of the last call


def kernel(**inputs):
    nc = _get_nc()
    in_maps = _in_maps(**{k: np.asarray(v) for k, v in inputs.items()})
    bkr = run_bass_kernel_spmd(nc, in_maps, core_ids=list(range(NCORES)))
    LAST_RESULTS[:] = [bkr]

    out = np.empty((B, L, D), dtype=np.float32)
    for core in range(NCORES):
        b, s = divmod(core, NSEG)
        out[b, s * SEG:(s + 1) * SEG, :] = bkr.results[core]["o"]
    return out
